# revision 40
# baseline (speedup 1.0000x reference)
"""3-layer GAT (4 heads x 64) + global mean pool + FC on 8 Trainium2 NeuronCores.

Strategy (graph-parallel):
  * Nodes are permuted and partitioned into 8 shards (one per core).  Within
    each core, nodes are tagged low/high (44%/56%, degree-alternating) and
    2D-greedily packed into 50 chunks of 128 dst nodes (low tags -> chunks
    0..21, high -> 22..49) balancing low- and high-half in-degree.
  * Per layer, each core computes the augmented projection
    h_aug = x @ [W | W@As | W@Ad] for its own node shard; shards are
    AllGathered into a replicated DRAM table (512B rows: 256 fp8e4m3 h values
    + 4 bf16 src attention scores).  The table row order is PIECE-major
    (4 pieces of 11/11/14/14 chunks x 8 cores) so the AllGather is split into
    4 chunked collectives that overlap the producing edge phase; only the
    last piece is exposed at the layer boundary.
  * Edges live on the core that owns their dst node, sorted by (chunk, src
    half, dst).  The table is split at row SPLIT=22528 (the low pieces) so
    int16 gather indices reach both halves; per-chunk slot budget is
    8 low + 10 high 128-slot chunks.
  * Per 128-edge chunk the kernel gathers 512B table rows with SWDGE indirect
    DMAs round-robined over 4 queues (Pool-engine descriptor generation and
    total DMA bytes are the bottlenecks), adds the dst attention score via a
    one-hot matmul (host-precomputed fp8 dst-major one-hot x per-chunk sd
    from SBUF), applies leaky-relu (fused DVE op) + exp (Activation engine),
    multiplies h by the per-edge weight out-of-place, and contracts
    numerator+denominator into one [128, 260] PSUM per chunk.
  * out = relu(num/den + b); residual after layer 3; each batch's postproc is
    software-pipelined one batch behind its aggregation, and the next layer's
    projection chunk is emitted inline so only the (last piece of the)
    AllGather sits between layers.  Graph mean-pool is a matmul against a
    host-built one-hot graph matrix + a tiny AllReduce; the final FC runs
    replicated on every core.
"""
import math
import os
import numpy as np

import concourse.bass as bass
import concourse.tile as tile
from concourse import bacc, mybir
from concourse.masks import make_identity

F32 = mybir.dt.float32
BF16 = mybir.dt.bfloat16
I32 = mybir.dt.int32
I16 = mybir.dt.int16
NPBF16 = mybir.dt.np(BF16)
FP8 = mybir.dt.float8e4
NPFP8 = mybir.dt.np(FP8)

AF = mybir.ActivationFunctionType
ALU = mybir.AluOpType
_NQ = int(os.environ.get('GAT_NQUEUES', '4'))
_SP = os.environ.get('GAT_SINGLE_PACKET', '0') == '1'


class Cfg:
    def __init__(self, N=50000, E=800000, IN=64, OUT=64, H=4, G=64, C=10,
                 NCORES=8, NCHUNK=50, LO_CHUNKS=28, neg_slope=0.2,
                 CPB_L=9, CPB_H=8, PIECES=(28, 22)):
        self.N, self.E, self.IN, self.OUT, self.H, self.G, self.C = N, E, IN, OUT, H, G, C
        self.HID = H * OUT                     # 256
        self.NCORES = NCORES
        self.NCHUNK = NCHUNK                   # dst chunks per core (128 nodes each)
        self.M = 128                           # dst nodes per chunk
        self.LO_CHUNKS = LO_CHUNKS             # chunks 0..LO_CHUNKS-1 hold low-tagged nodes
        self.NLOC = NCHUNK * self.M
        self.NPAD = NCORES * self.NLOC
        self.PAIRS = NCHUNK
        self.CPB_L = CPB_L                     # low slot-chunks per dst chunk
        self.CPB_H = CPB_H
        self.JB = CPB_L + CPB_H                # slot-chunks per batch (= dst chunk)
        self.JTOT = NCHUNK * self.JB
        self.NBATCH = NCHUNK
        self.PIECES = tuple(PIECES)            # chunk counts per AllGather piece
        assert sum(PIECES) == NCHUNK
        # piece k covers chunks [chunk0[k], chunk0[k]+PIECES[k]) of every core
        self.piece_chunk0 = [sum(PIECES[:k]) for k in range(len(PIECES))]
        # table row base of piece k (piece-major, cores interleaved per piece)
        self.piece_rowbase = [NCORES * self.M * c0 for c0 in self.piece_chunk0]
        self.SPLIT = NCORES * self.M * LO_CHUNKS   # 22528: all low pieces
        # low/high boundary must coincide with a piece boundary
        assert LO_CHUNKS in self.piece_chunk0 + [NCHUNK]
        self.neg_slope = neg_slope
        assert self.SPLIT <= 32768 and self.NPAD - self.SPLIT <= 32768
        # per-batch gather counts (immediates); preprocess overwrites these
        self.CNT_LO = tuple([CPB_L * 128] * NCHUNK)
        self.CNT_HI = tuple([CPB_H * 128] * NCHUNK)


# ----------------------------------------------------------------------------
# Host-side preprocessing
# ----------------------------------------------------------------------------

def _snake(n, nbins):
    i = np.arange(n)
    m = i % (2 * nbins)
    return np.where(m < nbins, m, 2 * nbins - 1 - m)


def _pack_blocks(dlo, dhi, nblocks, cap_lo, cap_hi, M):
    """Greedily assign nodes (with per-node low/high in-degree) to nblocks
    blocks of at most M nodes, balancing both dims against the caps."""
    order = np.argsort(-(dlo + dhi), kind="stable")
    bl = np.zeros(nblocks)
    bh = np.zeros(nblocks)
    cnt = np.zeros(nblocks, np.int64)
    blk = np.empty(len(dlo), np.int64)
    for n in order:
        score = np.maximum((bl + dlo[n]) / cap_lo, (bh + dhi[n]) / cap_hi)
        score[cnt >= M] = np.inf
        b = int(np.argmin(score))
        blk[n] = b
        bl[b] += dlo[n]
        bh[b] += dhi[n]
        cnt[b] += 1
    return blk, int(bl.max()), int(bh.max())


def preprocess(cfg, x, edge_index, batch, params):
    N = cfg.N
    NC, NLOC, M = cfg.NCORES, cfg.NLOC, cfg.M
    NCHUNK, LOC = cfg.NCHUNK, cfg.LO_CHUNKS
    src0 = np.asarray(edge_index[0], dtype=np.int64)
    dst0 = np.asarray(edge_index[1], dtype=np.int64)
    batch = np.asarray(batch, dtype=np.int64)
    x = np.asarray(x, dtype=np.float32)

    # self loops are NOT materialized as edges: their attention term is
    # computed analytically on-device from the locally kept h/ss/sd, so only
    # the E real edges are gathered
    deg = np.bincount(dst0, minlength=N)              # real in-degree
    order = np.argsort(-deg, kind="stable")           # nodes by in-degree desc

    # phase A: node -> core (degree-balanced snake)
    core_of_rank = _snake(N, NC)
    core_of = np.empty(N, dtype=np.int64)
    core_of[order] = core_of_rank

    # low/high tag: within each core, alternate by degree rank at LOC/NCHUNK
    low_node = np.zeros(N, dtype=bool)
    core_nodes = []                                   # degree-desc nodes per core
    for c in range(NC):
        nodes_c = order[core_of_rank == c]
        core_nodes.append(nodes_c)
        i = np.arange(len(nodes_c))
        lo_mask = (i * LOC // NCHUNK) != ((i + 1) * LOC // NCHUNK)
        assert lo_mask.sum() <= LOC * M and (~lo_mask).sum() <= (NCHUNK - LOC) * M
        low_node[nodes_c[lo_mask]] = True

    # per-node low/high in-degree
    dlow = np.bincount(dst0[low_node[src0]], minlength=N).astype(np.int64)
    dhigh = deg - dlow

    # phase B: node -> chunk within core, 2D-balanced greedy packing
    # (low-tagged nodes into chunks [0, LOC), high into [LOC, NCHUNK))
    CAP_L, CAP_H = cfg.CPB_L * 128, cfg.CPB_H * 128
    perm = np.empty(N, dtype=np.int64)
    maxlow = 0
    maxhigh = 0
    for c in range(NC):
        nodes_c = core_nodes[c]
        parts = [(nodes_c[low_node[nodes_c]], 0, LOC),
                 (nodes_c[~low_node[nodes_c]], LOC, NCHUNK - LOC)]
        for nodes_p, b0, nb in parts:
            assert len(nodes_p) <= nb * M
            blk, ml, mh = _pack_blocks(dlow[nodes_p], dhigh[nodes_p],
                                       nb, CAP_L, CAP_H, M)
            maxlow = max(maxlow, ml)
            maxhigh = max(maxhigh, mh)
            slot = np.zeros(len(nodes_p), dtype=np.int64)
            counts = np.zeros(nb, dtype=np.int64)
            for i in range(len(nodes_p)):
                b = blk[i]
                slot[i] = counts[b]
                counts[b] += 1
            perm[nodes_p] = c * NLOC + (b0 + blk) * M + slot

    cpb_l = max(cfg.CPB_L, math.ceil(maxlow / 128))
    cpb_h = max(cfg.CPB_H, math.ceil(maxhigh / 128))
    if (cpb_l, cpb_h) != (cfg.CPB_L, cfg.CPB_H):
        cfg = Cfg(N=cfg.N, E=cfg.E, IN=cfg.IN, OUT=cfg.OUT, H=cfg.H, G=cfg.G,
                  C=cfg.C, NCORES=cfg.NCORES, NCHUNK=cfg.NCHUNK,
                  LO_CHUNKS=cfg.LO_CHUNKS, neg_slope=cfg.neg_slope,
                  CPB_L=cpb_l, CPB_H=cpb_h, PIECES=cfg.PIECES)

    # piece-major table row numbering
    piece_of_chunk = np.repeat(np.arange(len(cfg.PIECES)), cfg.PIECES)
    pc0 = np.asarray(cfg.piece_chunk0, np.int64)
    prb = np.asarray(cfg.piece_rowbase, np.int64)
    pcn = np.asarray(cfg.PIECES, np.int64)

    def table_row(pos):
        "global node position (core*NLOC + p) -> table row (piece-major)"
        c = pos // NLOC
        p = pos % NLOC
        j = p // M
        k = piece_of_chunk[j]
        return prb[k] + c * (pcn[k] * M) + (j - pc0[k]) * M + (p % M)

    # ---- edge slot construction ----
    srcE = perm[src0]
    dstE = perm[dst0]
    rowE = table_row(srcE)
    lowE = rowE < cfg.SPLIT
    chunkE = dstE // M                                 # global chunk (core*NCHUNK + j)
    key = (chunkE * 2 + (~lowE)) * (cfg.NPAD + 1) + dstE
    o = np.argsort(key, kind="stable")
    rowE, dstE, lowE = rowE[o], dstE[o], lowE[o]

    grp = chunkE[o] * 2 + (~lowE)
    grp_counts = np.bincount(grp, minlength=NC * NCHUNK * 2)
    grp_start = np.concatenate([[0], np.cumsum(grp_counts)])[:-1]
    rank = np.arange(len(dstE)) - grp_start[grp]

    CL, CH, JB = cfg.CPB_L, cfg.CPB_H, cfg.JB
    nslot_core = NCHUNK * JB * 128
    core_e = dstE // NLOC
    j_e = (dstE % NLOC) // M
    base = core_e * nslot_core + j_e * (JB * 128)
    slot = np.where(lowE, base + rank, base + CL * 128 + rank)
    assert rank[lowE].max() < CL * 128 and rank[~lowE].max() < CH * 128

    nslots = NC * nslot_core
    hidx_slot = np.full(nslots, -1, dtype=np.int16)
    dloc_slot = np.full(nslots, -1, dtype=np.int64)    # dst row within core
    hidx_slot[slot] = np.where(lowE, rowE, rowE - cfg.SPLIT).astype(np.int16)
    dloc_slot[slot] = dstE % NLOC

    # Equalized gather counts: slots are rank-packed (real edges first per
    # (chunk, half) region), so the gather only needs to fetch up to the
    # across-core max real count per batch; that count is baked into the
    # program as the per-batch num_idxs_reg immediate.  Pad each core's
    # region with row-0 fetches up to the shared count (indices must be
    # non-negative below the count, and -1 beyond it).
    cnt2 = grp_counts.reshape(NC, NCHUNK, 2)
    cnt_lo = np.minimum(np.ceil(cnt2[:, :, 0].max(axis=0) / 16).astype(np.int64)
                        * 16, CL * 128)
    cnt_hi = np.minimum(np.ceil(cnt2[:, :, 1].max(axis=0) / 16).astype(np.int64)
                        * 16, CH * 128)
    for c in range(NC):
        for j in range(NCHUNK):
            b = c * nslot_core + j * (JB * 128)
            hidx_slot[b + cnt2[c, j, 0]:b + cnt_lo[j]] = 0
            b += CL * 128
            hidx_slot[b + cnt2[c, j, 1]:b + cnt_hi[j]] = 0
    cfg.CNT_LO = tuple(int(v) for v in cnt_lo)
    cfg.CNT_HI = tuple(int(v) for v in cnt_hi)

    JT = cfg.JTOT
    lowsel = np.zeros(nslot_core, dtype=bool)
    for j in range(NCHUNK):
        jb = j * JB * 128
        lowsel[jb:jb + CL * 128] = True

    # ---- weights ----
    W1, as1, ad1, b1 = params["W1"], params["as1"], params["ad1"], params["b1"]
    W2, as2, ad2, b2 = params["W2"], params["as2"], params["ad2"], params["b2"]
    W3, as3, ad3, b3 = params["W3"], params["as3"], params["ad3"], params["b3"]
    fcW, fcb = params["fcW"], params["fcb"]

    def aug(W, a_s, a_d):
        W = np.asarray(W, np.float32)
        HID, H, OUT = cfg.HID, cfg.H, cfg.OUT
        As = np.zeros((HID, H), np.float32)
        Ad = np.zeros((HID, H), np.float32)
        for h in range(H):
            As[h * OUT:(h + 1) * OUT, h] = np.asarray(a_s, np.float32)[h]
            Ad[h * OUT:(h + 1) * OUT, h] = np.asarray(a_d, np.float32)[h]
        return np.concatenate([W, W @ As, W @ Ad], axis=1)  # [in, HID+2H]

    w1a = aug(W1, as1, ad1)
    w2a = aug(W2, as2, ad2)
    w3a = aug(W3, as3, ad3)
    WA = cfg.HID + 2 * cfg.H                          # 264

    def pack_k(w):                                    # [256, WA] -> [128, 2*WA]
        return np.ascontiguousarray(
            w.reshape(2, 128, WA).transpose(1, 0, 2).reshape(128, 2 * WA))

    fcw_aug = np.concatenate([np.asarray(fcW, np.float32),
                              np.asarray(fcb, np.float32)[None, :]], axis=0)
    fcw_pad = np.zeros((384, cfg.C), np.float32)
    fcw_pad[:257] = fcw_aug
    fcw_m = np.ascontiguousarray(
        fcw_pad.reshape(3, 128, cfg.C).transpose(1, 0, 2).reshape(128, 3 * cfg.C))

    in_maps = []
    for c in range(NC):
        lo, hi = c * NLOC, (c + 1) * NLOC
        mask = (perm >= lo) & (perm < hi)
        origs = np.nonzero(mask)[0]
        locs = perm[origs] - lo
        xs = np.zeros((NLOC, cfg.IN), np.float32)
        xs[locs] = x[origs]
        og = np.zeros((NLOC, cfg.G), np.float32)
        og[locs, batch[origs]] = 1.0
        og_m = np.ascontiguousarray(
            og.reshape(cfg.NCHUNK, 128, cfg.G).transpose(1, 0, 2)
              .reshape(128, cfg.NCHUNK * cfg.G))
        hv = hidx_slot[c * nslot_core:(c + 1) * nslot_core]
        rep = lambda a: np.ascontiguousarray(np.tile(a.reshape(-1, 16).T, (8, 1)))
        # one-hots from the slot -> dst-row map
        dl = dloc_slot[c * nslot_core:(c + 1) * nslot_core]
        jj = np.arange(nslot_core) // 128              # chunk of each slot
        pp = np.arange(nslot_core) % 128               # partition of each slot
        valid = dl >= 0
        # slot-major [128, JTOT*128]: (p, j*128 + dst%128) — dst within pair
        o_ag = np.zeros((128, JT * 128), dtype=NPFP8)
        o_ag[pp[valid], jj[valid] * 128 + (dl[valid] % 128)] = 1.0
        # dst-major [128, JTOT*128]: (dst%128, j*128 + p)
        o_sd = np.zeros((128, JT * 128), dtype=NPFP8)
        o_sd[dl[valid] % 128, jj[valid] * 128 + pp[valid]] = 1.0
        in_maps.append({
            "xT": np.ascontiguousarray(xs.T).astype(NPBF16),
            "hidxl": rep(hv[lowsel]),
            "hidxh": rep(hv[~lowsel]),
            "oag": o_ag,
            "osd": o_sd,
            "og": og_m.astype(NPBF16),
            "w1": w1a.astype(NPBF16),
            "w2": pack_k(w2a).astype(NPBF16),
            "w3": pack_k(w3a).astype(NPBF16),
            "b1": np.asarray(b1, np.float32).reshape(1, cfg.HID),
            "b2": np.asarray(b2, np.float32).reshape(1, cfg.HID),
            "b3": np.asarray(b3, np.float32).reshape(1, cfg.HID),
            "fcw": fcw_m,
        })
    return cfg, in_maps


# ----------------------------------------------------------------------------
# Device program
# ----------------------------------------------------------------------------

def build_program(cfg, debug=False):
    nc = bacc.Bacc(None, target_bir_lowering=False, debug=debug,
                   num_devices=cfg.NCORES, num_swdge_queues=_NQ)
    HID, WA, H, OUT = cfg.HID, cfg.HID + 2 * cfg.H, cfg.H, cfg.OUT
    NLOC, NPAD, NCHUNK, PAIRS = cfg.NLOC, cfg.NPAD, cfg.NCHUNK, cfg.PAIRS
    JB, JTOT = cfg.JB, cfg.JTOT
    NBATCH = cfg.NBATCH
    CL, CH = cfg.CPB_L, cfg.CPB_H
    RG = [list(range(cfg.NCORES))]
    ROWW = 256  # table row in bf16 cols: h as 256xfp8 (cols 0:128) | ss bf16 (128:132) | pad
    HSS = 128  # bf16 col where ss starts
    NPIECE = len(cfg.PIECES)
    # piece k spans local hin rows [p_r0[k], p_r1[k]) and table rows
    # [piece_rowbase[k], +NCORES*piece rows)
    p_r0 = [c0 * cfg.M for c0 in cfg.piece_chunk0]
    p_r1 = [p_r0[k] + cfg.PIECES[k] * cfg.M for k in range(NPIECE)]
    # batch index after whose table_chunk the piece collective fires (with a
    # little slack so the Pool engine doesn't head-of-line block on the DMA)
    p_trig = [min(p_r1[k] // cfg.M + 1, NCHUNK - 1) for k in range(NPIECE)]

    d_xT = nc.dram_tensor("xT", [cfg.IN, NLOC], BF16, kind="ExternalInput")
    d_hidxl = nc.dram_tensor("hidxl", [128, NCHUNK * CL * 8], I16, kind="ExternalInput")
    d_hidxh = nc.dram_tensor("hidxh", [128, NCHUNK * CH * 8], I16, kind="ExternalInput")
    d_oag = nc.dram_tensor("oag", [128, JTOT * 128], FP8, kind="ExternalInput")
    d_osd = nc.dram_tensor("osd", [128, JTOT * 128], FP8, kind="ExternalInput")
    d_og = nc.dram_tensor("og", [128, NCHUNK * cfg.G], BF16, kind="ExternalInput")
    d_w1 = nc.dram_tensor("w1", [cfg.IN, WA], BF16, kind="ExternalInput")
    d_w2 = nc.dram_tensor("w2", [128, 2 * WA], BF16, kind="ExternalInput")
    d_w3 = nc.dram_tensor("w3", [128, 2 * WA], BF16, kind="ExternalInput")
    d_b = [nc.dram_tensor(f"b{i}", [1, HID], F32, kind="ExternalInput")
           for i in (1, 2, 3)]
    d_fcw = nc.dram_tensor("fcw", [128, 3 * cfg.C], F32, kind="ExternalInput")
    d_out = nc.dram_tensor("out", [cfg.G, cfg.C], F32, kind="ExternalOutput")

    with tile.TileContext(nc, num_cores=cfg.NCORES) as tc:
        dram = tc.alloc_tile_pool(name="dram", bufs=1, space="DRAM")
        consts = tc.alloc_tile_pool(name="consts", bufs=1)
        stage = tc.alloc_tile_pool(name="stage", bufs=3)
        xtp = tc.alloc_tile_pool(name="xtp", bufs=1)
        wp = tc.alloc_tile_pool(name="wp", bufs=2)
        ep = tc.alloc_tile_pool(name="ep", bufs=3)
        pp = tc.alloc_tile_pool(name="pp", bufs=3)
        sp = tc.alloc_tile_pool(name="sp", bufs=2)
        ps_h = tc.alloc_tile_pool(name="ps_h", bufs=1, space="PSUM")
        ps_pair = tc.alloc_tile_pool(name="ps_pair", bufs=2, space="PSUM")
        ps_sd = tc.alloc_tile_pool(name="ps_sd", bufs=2, space="PSUM")
        ps_t = tc.alloc_tile_pool(name="ps_t", bufs=2, space="PSUM")
        ps_misc = tc.alloc_tile_pool(name="ps_misc", bufs=1, space="PSUM")

        # --- DRAM scratch ---
        hin_h = dram.tile([NLOC, ROWW], BF16)
        # One Shared tensor per (layer, piece): the sim allows only a single
        # writer inst per Shared tensor, so each piece collective gets its
        # own.  Piece 0 is the low half (gather-low source), piece 1 high.
        # One pad row each: gathers of the last row read 768B from a
        # 528B-used row.
        p_rows = [cfg.NCORES * (p_r1[k] - p_r0[k]) for k in range(NPIECE)]
        tbls_p = [[dram.tile([p_rows[k] + 1, ROWW], BF16, addr_space="Shared",
                             name=f"tbl{i}_{k}") for k in range(NPIECE)]
                  for i in range(3)]
        pool_in = dram.tile([cfg.G, HID + 1], F32)
        pool_out = dram.tile([cfg.G, HID + 1], F32, addr_space="Shared")

        # --- resident constants ---
        # (projection-critical tensors first so layer-1 table chunks and the
        # first AllGather piece can start before the big index tables land)
        s_xT1 = xtp.tile([cfg.IN, NLOC], BF16, tag="xt")
        nc.sync.dma_start(out=s_xT1[:], in_=d_xT[:, :])
        s_w1 = consts.tile([cfg.IN, WA], BF16)
        nc.sync.dma_start(out=s_w1[:], in_=d_w1[:, :])

        s_hidxl = consts.tile([128, NCHUNK * CL * 8], I16)
        s_hidxh = consts.tile([128, NCHUNK * CH * 8], I16)
        s_og = consts.tile([128, NCHUNK, cfg.G], BF16)
        nc.sync.dma_start(out=s_hidxl[:], in_=d_hidxl[:, :])
        nc.sync.dma_start(out=s_hidxh[:], in_=d_hidxh[:, :])
        nc.sync.dma_start(out=s_og[:], in_=d_og[:, :].rearrange("p (i g) -> p i g", g=cfg.G))
        s_w2 = consts.tile([128, 2, WA], BF16)
        nc.sync.dma_start(out=s_w2[:], in_=d_w2[:, :].rearrange("p (k w) -> p k w", k=2))
        s_w3 = consts.tile([128, 2, WA], BF16)
        nc.sync.dma_start(out=s_w3[:], in_=d_w3[:, :].rearrange("p (k w) -> p k w", k=2))
        s_fcw = consts.tile([128, 3, cfg.C], F32)
        nc.sync.dma_start(out=s_fcw[:], in_=d_fcw[:, :].rearrange("p (k c) -> p k c", k=3))

        ident_bf = consts.tile([128, 128], BF16)
        make_identity(nc, ident_bf[:])
        ident_f32 = consts.tile([128, 128], F32)
        make_identity(nc, ident_f32[:])
        ones_row = consts.tile([1, cfg.G], F32)
        nc.vector.memset(ones_row[:], 1.0)

        x1_res = consts.tile([128, NCHUNK, HID], BF16)   # layer-1 activations

        # gather landing buffers, rotated manually; zeroed once so that slots
        # beyond the equalized gather count never expose uninitialized SBUF
        # (NaN x one-hot-zero would still poison the aggregation PSUM)
        NGT = 6
        gts = [consts.tile([128, JB, ROWW], BF16, name=f"gtbuf{i}",
                           tag=f"gtbuf{i}") for i in range(NGT)]
        for t in gts:
            nc.vector.memset(t[:], 0.0)

        bias_ts = []
        for i in range(3):
            bt = consts.tile([128, HID], F32, name=f"bias{i}", tag=f"bias{i}")
            nc.sync.dma_start(out=bt[:], in_=bass.AP(
                tensor=d_b[i][:, :].tensor, offset=0, ap=[[0, 128], [1, HID]]))
            bias_ts.append(bt)

        sdb_ref = [None, None]  # (sdb, ssb) tiles for current / next layer
        # local h of the core's own chunks (fp8), for the analytic self-loop
        # term: postproc(L, g) reads slot g just before table_chunk(L+1, g)
        # overwrites it, so a single buffer suffices
        h_loc = consts.tile([128, NCHUNK, HID], FP8, name="h_loc")

        def table_chunk(layer, i, xT_t, w_t, khalves, sdb, ssb):
            """Emit projection of node-chunk i into table `layer` + sd/ss/h
            capture for the self-loop term."""
            ph = ps_h.tile([128, WA], F32, name=f"ph{layer}_{i}", tag="ph")
            for k in range(khalves):
                if khalves == 1:
                    lhsT = xT_t[:, i * 128:(i + 1) * 128]
                    rhs = w_t[:, :]
                else:
                    lhsT = xT_t[:, k, i * 128:(i + 1) * 128]
                    rhs = w_t[:, k, :]
                nc.tensor.matmul(out=ph[:], lhsT=lhsT, rhs=rhs,
                                 start=(k == 0), stop=(k == khalves - 1))
            h_st = stage.tile([128, HSS + H], BF16, name=f"hst{layer}_{i}", tag="hst")
            nc.scalar.copy(out=h_st[:, 0:HSS].bitcast(FP8), in_=ph[:, 0:HID])
            nc.scalar.copy(out=h_loc[:, i, :], in_=ph[:, 0:HID])
            nc.vector.tensor_copy(out=h_st[:, HSS:HSS + H], in_=ph[:, HID:HID + H])
            nc.vector.tensor_copy(out=ssb[:, i, :], in_=ph[:, HID:HID + H])
            nc.vector.tensor_copy(out=sdb[:, i, :], in_=ph[:, HID + H:WA])
            nc.sync.dma_start(
                out=hin_h[i * 128:(i + 1) * 128, 0:HSS + H], in_=h_st[:])

        def gather_piece(layer, k):
            """AllGather piece k of table `layer` (rows p_r0[k]:p_r1[k] of
            every core's hin_h).  Only the used 264B of each 512B row move
            (strided in/out APs); the table keeps the 512B row stride the
            gathers need."""
            tbl = tbls_p[layer][k]
            r0, r1 = p_r0[k], p_r1[k]
            nc.gpsimd.collective_compute(
                "AllGather", ALU.bypass, replica_groups=RG,
                ins=[hin_h[r0:r1, :].opt()],
                outs=[tbl[0:p_rows[k], :].opt()])

        def edge_phase(layer, bias_t, xT_next, w_next, ps_pool_t):
            tbl_lo, tbl_hi = tbls_p[layer][0], tbls_p[layer][1]
            sdb, ssb = sdb_ref[0]
            nxt = sdb_ref[1]
            nL, nH = CL * 128, CH * 128
            # batch after whose table_chunk each piece of the NEXT layer's
            # table is AllGathered (piece p covers chunks < p_trig[p])
            trig_of_batch = {}
            if xT_next is not None:
                for k in range(NPIECE):
                    trig_of_batch.setdefault(p_trig[k], []).append(k)
            prev_ps = [None]

            def flush_prev(last=False):
                if prev_ps[0] is None:
                    return
                pg, pps = prev_ps[0]
                postproc(layer, pg, pps, bias_t, xT_next, ps_pool_t, sdb, ssb)
                if xT_next is not None:
                    table_chunk(layer + 1, pg, xT_next, w_next, 2, *nxt)
                    pieces = list(trig_of_batch.get(pg, ()))
                    if last:
                        pieces += [k for k in range(NPIECE) if p_trig[k] > pg]
                    for k in pieces:
                        gather_piece(layer + 1, k)
                prev_ps[0] = None

            def emit_lo(g):
                # low-half gather for batch g; emitted LA batches ahead of the
                # high-half one so the Pool engine generates low descriptors
                # while the (exposed) high-half table collective finishes, and
                # a waiting high gather never head-of-line blocks low ones
                nc.gpsimd.dma_gather(
                    out_ap=gts[g % NGT][:, 0:CL, :], in_ap=tbl_lo[0:cfg.SPLIT, :],
                    idxs_ap=s_hidxl[:, g * (nL // 16):(g + 1) * (nL // 16)],
                    num_idxs=nL, num_idxs_reg=cfg.CNT_LO[g], elem_size=ROWW,
                    single_packet=_SP, queue_num=(2 * g) % _NQ)

            LA = 4
            for g in range(min(LA, NBATCH)):
                emit_lo(g)
            GM = 2                     # batches per merged one-hot load
            for gg in range(NBATCH // GM):
                g0 = gg * GM
                j0 = g0 * JB
                # one-hot loads (independent of the table -> prefetch freely)
                oag = ep.tile([128, GM * JB, 128], FP8, name=f"oag{layer}_{gg}",
                              tag="oag", bufs=2)
                nc.sync.dma_start(out=oag[:],
                                  in_=d_oag[:, j0 * 128:(j0 + GM * JB) * 128]
                                  .rearrange("p (j m) -> p j m", m=128))
                osd = ep.tile([128, GM * JB, 128], FP8, name=f"osd{layer}_{gg}",
                              tag="osd", bufs=2)
                nc.sync.dma_start(out=osd[:],
                                  in_=d_osd[:, j0 * 128:(j0 + GM * JB) * 128]
                                  .rearrange("p (j m) -> p j m", m=128))
                for t in range(GM):
                    g = g0 + t
                    co = t * JB        # chunk offset in the merged one-hots
                    gt = gts[g % NGT]
                    if g + LA < NBATCH:
                        emit_lo(g + LA)
                    nc.gpsimd.dma_gather(
                        out_ap=gt[:, CL:JB, :], in_ap=tbl_hi[0:NPAD - cfg.SPLIT, :],
                        idxs_ap=s_hidxh[:, g * (nH // 16):(g + 1) * (nH // 16)],
                        num_idxs=nH, num_idxs_reg=cfg.CNT_HI[g], elem_size=ROWW,
                        single_packet=_SP, queue_num=(2 * g + 1) % _NQ)
                    # per-slot dst score: one-hot^T @ sd (independent of gather)
                    sdps = ps_sd.tile([128, JB, H], F32, name=f"sdps{layer}_{g}",
                                      tag="sdps")
                    for c in range(JB):
                        nc.tensor.matmul(out=sdps[:, c, :], lhsT=osd[:, co + c, :],
                                         rhs=sdb[:, g, :], start=True, stop=True)
                    # e = lrelu(ss + sd) = (e*slope) max e; alpha = exp(e)
                    e_t = ep.tile([128, JB, H], F32, name=f"e{layer}_{g}", tag="e")
                    nc.vector.tensor_tensor(out=e_t[:], in0=gt[:, :, HSS:HSS + H],
                                            in1=sdps[:], op=ALU.add)
                    el = ep.tile([128, JB, H], F32, name=f"el{layer}_{g}", tag="el")
                    nc.vector.scalar_tensor_tensor(
                        out=el[:], in0=e_t[:], scalar=cfg.neg_slope, in1=e_t[:],
                        op0=ALU.mult, op1=ALU.max)
                    # alpha -> gtw den cols; alpha-weighted h -> gtw num cols
                    gtw = ep.tile([128, JB, WA - H], BF16, name=f"gtw{layer}_{g}",
                                  tag="gtw", bufs=3)
                    nc.scalar.activation(out=gtw[:, :, HID:HID + H], in_=el[:],
                                         func=AF.Exp)
                    nc.vector.tensor_tensor(
                        out=gtw[:, :, 0:HID].rearrange("p a (h o) -> p a h o", o=OUT),
                        in0=gt[:, :, 0:HSS].bitcast(FP8)
                            .rearrange("p a (h o) -> p a h o", o=OUT),
                        in1=gtw[:, :, HID:HID + H].to_broadcast([128, JB, H, OUT]),
                        op=ALU.mult)
                    blk_ps = ps_pair.tile([128, WA - H], F32, name=f"pp{layer}_{g}",
                                          tag="pp")
                    for c in range(JB):
                        nc.tensor.matmul(
                            out=blk_ps[:], lhsT=oag[:, co + c, :],
                            rhs=gtw[:, c, :],
                            start=(c == 0), stop=(c == JB - 1))
                    # software pipeline: postproc of the PREVIOUS batch, so no
                    # queued vector op waits on this batch's aggregation matmuls
                    flush_prev()
                    prev_ps[0] = (g, blk_ps)
            flush_prev(last=True)

        def postproc(layer, pair, ppz, bias_t, xT_next, ps_pool_t, sdb, ssb):
            # analytic self-loop term: alpha_s = exp(lrelu(ss_v + sd_v))
            es = pp.tile([128, 2 * H], F32, name=f"es{layer}_{pair}", tag="es")
            nc.vector.tensor_tensor(out=es[:, 0:H], in0=ssb[:, pair, :],
                                    in1=sdb[:, pair, :], op=ALU.add)
            nc.vector.scalar_tensor_tensor(
                out=es[:, H:2 * H], in0=es[:, 0:H], scalar=cfg.neg_slope,
                in1=es[:, 0:H], op0=ALU.mult, op1=ALU.max)
            als = pp.tile([128, H], F32, name=f"als{layer}_{pair}", tag="als")
            nc.scalar.activation(out=als[:], in_=es[:, H:2 * H], func=AF.Exp)
            den = pp.tile([128, H], F32, name=f"den{layer}_{pair}", tag="den")
            nc.vector.tensor_tensor(out=den[:], in0=ppz[:, HID:HID + H],
                                    in1=als[:], op=ALU.add)
            nc.vector.reciprocal(den[:], den[:])
            # numerator += alpha_s * h_v, then normalize and add bias
            xf = pp.tile([128, HID], F32, name=f"xf{layer}_{pair}", tag="xf")
            nc.vector.tensor_tensor(
                out=xf[:].rearrange("p (h o) -> p h o", o=OUT),
                in0=h_loc[:, pair, :].rearrange("p (h o) -> p h o", o=OUT),
                in1=als[:].to_broadcast([128, H, OUT]),
                op=ALU.mult)
            nc.vector.tensor_tensor(out=xf[:], in0=xf[:], in1=ppz[:, 0:HID],
                                    op=ALU.add)
            nc.vector.tensor_tensor(
                out=xf[:].rearrange("p (h o) -> p h o", o=OUT),
                in0=xf[:].rearrange("p (h o) -> p h o", o=OUT),
                in1=den[:].to_broadcast([128, H, OUT]),
                op=ALU.mult)
            nc.vector.tensor_tensor(out=xf[:], in0=xf[:], in1=bias_t[:], op=ALU.add)
            if layer == 0:
                xb = x1_res[:, pair, :]
            else:
                xb = pp.tile([128, HID], BF16, name=f"xb{layer}_{pair}", tag="xb")
            nc.scalar.activation(out=xb, in_=xf[:], func=AF.Relu)
            if layer < 2:
                for k in (0, 1):
                    pt = ps_t.tile([128, 128], BF16, name=f"pt{layer}_{pair}_{k}", tag="pt")
                    nc.tensor.transpose(out=pt[:], in_=xb[:, k * 128:(k + 1) * 128],
                                        identity=ident_bf[:])
                    nc.scalar.copy(
                        out=xT_next[:, k, pair * 128:(pair + 1) * 128], in_=pt[:])
            else:
                xr = pp.tile([128, HID + 1], BF16, name=f"xr{pair}", tag="xr")
                nc.vector.memset(xr[:, HID:HID + 1], 1.0)
                nc.vector.tensor_tensor(out=xr[:, 0:HID], in0=xb,
                                        in1=x1_res[:, pair, :], op=ALU.add)
                nc.tensor.matmul(out=ps_pool_t[:], lhsT=s_og[:, pair, :],
                                 rhs=xr[:], start=(pair == 0),
                                 stop=(pair == PAIRS - 1))

        # ---------------- main flow ----------------
        ps_pool_t = ps_misc.tile([cfg.G, HID + 1], F32, tag="misc")

        sdb1 = wp.tile([128, NCHUNK, H], BF16, name="sdb1", tag="sdb")
        ssb1 = wp.tile([128, NCHUNK, H], BF16, name="ssb1", tag="ssb")
        next_piece = 0
        for i in range(NCHUNK):
            table_chunk(0, i, s_xT1, s_w1, 1, sdb1, ssb1)
            while (next_piece < NPIECE
                   and i + 1 >= p_r1[next_piece] // cfg.M):
                gather_piece(0, next_piece)
                next_piece += 1

        xT2 = xtp.tile([128, 2, NLOC], BF16, name="xT2", tag="xt")
        sdb2 = wp.tile([128, NCHUNK, H], BF16, name="sdb2", tag="sdb")
        ssb2 = wp.tile([128, NCHUNK, H], BF16, name="ssb2", tag="ssb")
        sdb_ref[0], sdb_ref[1] = (sdb1, ssb1), (sdb2, ssb2)
        edge_phase(0, bias_ts[0], xT2, s_w2, None)

        xT3 = xtp.tile([128, 2, NLOC], BF16, name="xT3", tag="xt")
        sdb3 = wp.tile([128, NCHUNK, H], BF16, name="sdb3", tag="sdb")
        ssb3 = wp.tile([128, NCHUNK, H], BF16, name="ssb3", tag="ssb")
        sdb_ref[0], sdb_ref[1] = (sdb2, ssb2), (sdb3, ssb3)
        edge_phase(1, bias_ts[1], xT3, s_w3, None)

        sdb_ref[0], sdb_ref[1] = (sdb3, ssb3), None
        edge_phase(2, bias_ts[2], None, None, ps_pool_t)

        # ---------------- epilogue ----------------
        pl = sp.tile([cfg.G, HID + 1], F32)
        nc.vector.tensor_copy(pl[:], ps_pool_t[:])
        nc.sync.dma_start(out=pool_in[:, :], in_=pl[:])
        nc.gpsimd.collective_compute(
            "AllReduce", ALU.add, replica_groups=RG,
            ins=[pool_in[:].opt()], outs=[pool_out[:].opt()])
        pr = sp.tile([cfg.G, HID + 1], F32)
        nc.sync.dma_start(out=pr[:], in_=pool_out[:, :])
        cnt = sp.tile([cfg.G, 1], F32)
        nc.vector.tensor_scalar(out=cnt[:], in0=pr[:, HID:HID + 1],
                                scalar1=1.0, scalar2=None, op0=ALU.max)
        nc.vector.reciprocal(cnt[:], cnt[:])
        pa = sp.tile([cfg.G, HID + 1], F32)
        nc.vector.tensor_scalar(out=pa[:, 0:HID], in0=pr[:, 0:HID],
                                scalar1=cnt[:, 0:1], scalar2=None, op0=ALU.mult)
        nc.vector.memset(pa[:, HID:HID + 1], 1.0)
        paT = sp.tile([128, 2, cfg.G], F32)
        for k in (0, 1):
            pt = ps_t.tile([128, 128], F32, name=f"ptfc{k}", tag="pt")
            nc.tensor.transpose(out=pt[:, 0:cfg.G],
                                in_=pa[:, k * 128:(k + 1) * 128],
                                identity=ident_f32[0:cfg.G, 0:cfg.G])
            nc.vector.tensor_copy(paT[:, k, :], pt[:, 0:cfg.G])
        pfc = ps_misc.tile([cfg.G, cfg.C], F32, tag="misc")
        nc.tensor.matmul(out=pfc[:], lhsT=paT[:, 0, :], rhs=s_fcw[:, 0, :],
                         start=True, stop=False)
        nc.tensor.matmul(out=pfc[:], lhsT=paT[:, 1, :], rhs=s_fcw[:, 1, :],
                         start=False, stop=False)
        nc.tensor.matmul(out=pfc[:], lhsT=ones_row[:], rhs=s_fcw[0:1, 2, :],
                         start=False, stop=True)
        outt = sp.tile([cfg.G, cfg.C], F32)
        nc.vector.tensor_copy(outt[:], pfc[:])
        nc.sync.dma_start(out=d_out[:, :], in_=outt[:])

        for _pool in (ps_misc, ps_t, ps_sd, ps_pair, ps_h, sp, pp, ep, wp, xtp,
                      stage, consts, dram):
            _pool.release()

    nc.compile()
    return nc


# ----------------------------------------------------------------------------
# Entry point
# ----------------------------------------------------------------------------

_CACHE = {}


def _get_program(cfg):
    key = (cfg.N, cfg.NCHUNK, cfg.CPB_L, cfg.CPB_H, cfg.NCORES,
           cfg.LO_CHUNKS, cfg.PIECES, cfg.CNT_LO, cfg.CNT_HI)
    if key not in _CACHE:
        _CACHE[key] = build_program(cfg)
    return _CACHE[key]


def kernel(x, edge_index, batch, W1, as1, ad1, b1, W2, as2, ad2, b2,
           W3, as3, ad3, b3, fcW, fcb):
    from concourse.bass_utils import run_bass_kernel_spmd
    cfg = Cfg()
    params = dict(W1=W1, as1=as1, ad1=ad1, b1=b1, W2=W2, as2=as2, ad2=ad2,
                  b2=b2, W3=W3, as3=as3, ad3=ad3, b3=b3, fcW=fcW, fcb=fcb)
    cfg, in_maps = preprocess(cfg, x, edge_index, batch, params)
    nc = _get_program(cfg)
    res = run_bass_kernel_spmd(nc, in_maps, core_ids=list(range(cfg.NCORES)))
    return np.asarray(res.results[0]["out"], dtype=np.float32)
